# revision 1
# baseline (speedup 1.0000x reference)
"""Trainium2 Bass kernel: SENSE-CG MRI reconstruction (nn_CG_module).

Problem: 10 CG iterations (plus initial residual) of
    (A^H A + lam I) x = adj + lam x0
where A = mask * FFT2 * coil-maps (12 coils, 384x384, ortho FFTs).

Strategy (per NeuronCore, pure batch-parallel over 8 cores):
  - 2D FFTs as dense DFT matmuls on the TensorEngine. Each 1D stage
    computes out[j,k] = sum_r src[r,j] * G[r,k] with the *data* as the
    stationary operand, which yields the transposed partial for free;
    two stages restore orientation, so the whole fft2 -> mask -> ifft2
    chain needs zero transposes.
  - Complex planes stored COMPONENT-BLOCKED [128, 2304]: real block
    [0:1152] then imag block [1152:2304]; each block is 3 row-chunks of
    384 (row = rb*128 + partition). Every matmul stationary chunk and
    every elementwise product is contiguous.
  - Coil loop streams maps from HBM (never all resident), software-
    pipelined two ways: coil c+1's products are emitted before coil c's
    stages, and the PE windows interleave coil c-1's (S3,S4) with coil
    c's (S1,S2) group-by-group so PSUM evacuation latency never idles
    the PE.
  - Elementwise complex products split GPSIMD/VectorE, PSUM evacuation
    on ScalarE, CG reductions via scalar_tensor_tensor accum +
    gpsimd.partition_all_reduce.
"""
import numpy as np

B, C, H, W = 8, 12, 384, 384
LAM = 0.05
CG_MAX = 10
EPS = 1e-12
P = 128
NB = H // P            # 3 row blocks
FREE = NB * W          # 1152
FREEI = 2 * FREE       # 2304

_CACHE = {}


def build(cg_iters=CG_MAX, ncoils=C, mmdt="f32r", explicit_ldw=False,
          bufs=None):
    import concourse.mybir as mybir
    import concourse.tile as tile
    import concourse.bass_isa as bass_isa
    from concourse import bacc

    dt = mybir.dt
    Alu = mybir.AluOpType
    MDT = {"f32r": dt.float32r, "f16": dt.float16, "bf16": dt.bfloat16}[mmdt]

    nc = bacc.Bacc("TRN2", target_bir_lowering=False, debug=False)

    x_d = nc.dram_tensor("x", [H, W, 2], dt.float32, kind="ExternalInput")
    adj_d = nc.dram_tensor("adj", [H, W, 2], dt.float32, kind="ExternalInput")
    maps_d = nc.dram_tensor("maps", [ncoils, H, W, 2], dt.float32,
                            kind="ExternalInput")
    mask_d = nc.dram_tensor("mask", [H, W], dt.float32, kind="ExternalInput")
    gr_d = nc.dram_tensor("gr", [H, W], dt.float32, kind="ExternalInput")
    gi_d = nc.dram_tensor("gi", [H, W], dt.float32, kind="ExternalInput")
    out_d = nc.dram_tensor("out", [H, W, 2], dt.float32, kind="ExternalOutput")

    def dram_comp(t, comp, rb):  # [H,W,2] -> [p, w] one component/row-block
        return t.rearrange("(rb p) w two -> p rb w two", p=P)[:, rb, :, comp]

    def dram_iv(t):            # [H,W,2] -> [p, rb, (w two)]
        return t.rearrange("(rb p) w two -> p rb (w two)", p=P)

    def ivm(t):                # interleaved plane tile -> [p, rb, w, two]
        return t[:].rearrange("p (rb w two) -> p rb w two", rb=NB, two=2)

    def dram_rv(t):            # [H,W] -> [p, rb, w]
        return t.rearrange("(rb p) w -> p rb w", p=P)

    def cb(t, comp):           # comp block of a plane tile -> [p, rb, w]
        return t[:][:, comp * FREE:(comp + 1) * FREE].rearrange(
            "p (rb w) -> p rb w", rb=NB)

    def rv3(t):                # real scratch tile -> [p, rb, w]
        return t[:].rearrange("p (rb w) -> p rb w", rb=NB)

    bu = {"mp": 4, "ck": 4, "ws": 3, "im": 2, "up": 6}
    if bufs:
        bu.update(bufs)
    with tile.TileContext(nc) as tc:
        with (
            tc.tile_pool(name="state", bufs=1) as st,
            tc.tile_pool(name="gpool", bufs=1) as gp,
            tc.tile_pool(name="mapsp", bufs=bu["mp"]) as mp,
            tc.tile_pool(name="cimk", bufs=bu["ck"]) as ckp,
            tc.tile_pool(name="ws", bufs=bu["ws"]) as wp,
            tc.tile_pool(name="imp", bufs=bu["im"]) as imp,
            tc.tile_pool(name="up", bufs=bu["up"]) as up,
            tc.tile_pool(name="scal", bufs=4) as scp,
            tc.tile_pool(name="ps", bufs=4, space="PSUM") as ps,
        ):
            # ---------- persistent state (all comp-blocked) ----------
            xk = st.tile([P, FREEI], dt.float32, tag="xk")
            r_pl = st.tile([P, FREEI], dt.float32, tag="r")
            p_pl = st.tile([P, FREEI], dt.float32, tag="p")
            acc = st.tile([P, FREEI], dt.float32, tag="acc")
            gr_t = gp.tile([P, FREE], MDT, tag="gr")
            gi_t = gp.tile([P, FREE], MDT, tag="gi")
            gin_t = gp.tile([P, FREE], MDT, tag="gin")
            mask_t = gp.tile([P, FREE], dt.float32, tag="mask")
            rd_a = st.tile([P, 1], dt.float32, tag="rd_a")
            rd_b = st.tile([P, 1], dt.float32, tag="rd_b")

            # ---------- constants ----------
            for src_d, dst_t, scale in ((gr_d, gr_t, 1.0), (gi_d, gi_t, 1.0),
                                        (gi_d, gin_t, -1.0)):
                stg = up.tile([P, FREE], dt.float32, tag="u")
                nc.sync.dma_start(rv3(stg), dram_rv(src_d[:]))
                if scale == 1.0:
                    nc.scalar.copy(dst_t[:], stg[:])
                else:
                    nc.scalar.mul(dst_t[:], stg[:], scale)
            nc.sync.dma_start(rv3(mask_t), dram_rv(mask_d[:]))

            # ---------- DFT stage emitter ----------
            def stage(src, dst, fwd, out_mode):
                """dst[j,k] = sum_r src[r,j] * Gc[r,k]  (comp-blocked planes).

                fwd: Gc = G (forward DFT);  else Gc = conj(G).
                out_mode: 'plane' ACT copy (rounds to dst dtype),
                          'mask' DVE multiply by mask row-chunk.
                Returns NB closures (one per output group jb) so the caller
                controls PE interleaving across stages/coils.
                """
                if fwd:
                    gA, gB, gC, gD = gr_t, gin_t, gi_t, gr_t
                else:
                    gA, gB, gC, gD = gr_t, gi_t, gin_t, gr_t
                # dst viewed [p, two, rb, w] (comp-blocked)
                dvb = dst[:].rearrange("p (two rb w) -> p two rb w",
                                       two=2, rb=NB)

                def emit_group(jb):
                    # r-half in psum bank 0 ([0:384]), i-half in bank 1
                    # ([512:896]); rb-major order so the latest-arriving src
                    # chunk (rb=2) is consumed at MM 9/12 — its evacuation
                    # latency is covered by 8 earlier matmuls.
                    pt = ps.tile([P, 1024], dt.float32, tag="ps")
                    nhalf = {0: 0, 512: 0}
                    for rb in range(NB):
                        for off, comp, gx in ((0, 0, gA), (0, 1, gB),
                                              (512, 0, gC), (512, 1, gD)):
                            j = nhalf[off]
                            lhsT = src[:][:, comp * FREE + rb * W + jb * P:
                                          comp * FREE + rb * W + jb * P + P]
                            if explicit_ldw:
                                nc.tensor.ldweights(lhsT)
                            nc.tensor.matmul(
                                pt[:, off:off + W], lhsT,
                                gx[:, rb * W:(rb + 1) * W],
                                start=(j == 0), stop=(j == 2 * NB - 1))
                            nhalf[off] += 1
                    pin = pt[:].rearrange("p (two x) -> p two x", two=2)[:, :, 0:W]
                    dslice = dvb[:, :, jb, :]          # [p, 2, 384]
                    if out_mode == "mask":
                        mrow = mask_t[:, jb * W:(jb + 1) * W]
                        nc.vector.tensor_tensor(
                            dslice, pin,
                            mrow.unsqueeze(1).broadcast_to([P, 2, W]), Alu.mult)
                    else:
                        nc.scalar.copy(dslice, pin)

                return [lambda jb=jb: emit_group(jb) for jb in range(NB)]

            # ---------- SENSE normal operator: acc = A^H A v ----------
            def emit_normal(v_pl):
                mbs = [None] * ncoils
                cims = [None] * ncoils
                kpls = [None] * ncoils
                ims = [None] * ncoils

                def fetch_cim(c):
                    mb = mp.tile([P, FREEI], dt.float32, tag="mapsbuf")
                    nc.sync.dma_start(ivm(mb)[:, :, :, :].rearrange(
                        "p rb w two -> p rb (w two)"), dram_iv(maps_d[:][c]))
                    mbv = ivm(mb)
                    u = []
                    for ia, ib in ((0, 0), (1, 1), (0, 1), (1, 0)):
                        ut = up.tile([P, FREE], dt.float32, tag="u")
                        nc.gpsimd.tensor_tensor(rv3(ut), mbv[:, :, :, ia],
                                                cb(v_pl, ib), Alu.mult)
                        u.append(ut)
                    cim = ckp.tile([P, FREEI], MDT, tag="cimk")
                    nc.vector.tensor_tensor(cb(cim, 0), rv3(u[0]), rv3(u[1]),
                                            Alu.subtract)
                    nc.vector.tensor_tensor(cb(cim, 1), rv3(u[2]), rv3(u[3]),
                                            Alu.add)
                    mbs[c], cims[c] = mb, cim

                def make_A(c):
                    w1 = wp.tile([P, FREEI], MDT, tag="w1w3")
                    g1 = stage(cims[c], w1, fwd=True, out_mode="plane")
                    kpl = ckp.tile([P, FREEI], MDT, tag="cimk")
                    kpls[c] = kpl
                    g2 = stage(w1, kpl, fwd=True, out_mode="mask")
                    return g1 + g2

                def make_B(c):
                    w3 = wp.tile([P, FREEI], MDT, tag="w1w3")
                    g3 = stage(kpls[c], w3, fwd=False, out_mode="plane")
                    im = imp.tile([P, FREEI], dt.float32, tag="im")
                    ims[c] = im
                    g4 = stage(w3, im, fwd=False, out_mode="plane")
                    return g3 + g4

                def emit_accum(c):
                    # acc += conj(m)*im  (deprioritized: fills engine idle
                    # time, never preempts ops feeding the PE)
                    mbv, im = ivm(mbs[c]), ims[c]
                    with tc.high_priority(offset=-1_000_000):
                        t = []
                        for eng, ia, ib in ((nc.gpsimd, 0, 0), (nc.gpsimd, 1, 1),
                                            (nc.vector, 0, 1), (nc.vector, 1, 0)):
                            ut = up.tile([P, FREE], dt.float32, tag="u")
                            eng.tensor_tensor(rv3(ut), mbv[:, :, :, ia],
                                              cb(im, ib), Alu.mult)
                            t.append(ut)
                        if c == 0:
                            nc.vector.tensor_tensor(cb(acc, 0), rv3(t[0]),
                                                    rv3(t[1]), Alu.add)
                            nc.vector.tensor_tensor(cb(acc, 1), rv3(t[2]),
                                                    rv3(t[3]), Alu.subtract)
                        else:
                            nc.vector.tensor_tensor(cb(acc, 0), cb(acc, 0),
                                                    rv3(t[0]), Alu.add)
                            nc.vector.tensor_tensor(cb(acc, 0), cb(acc, 0),
                                                    rv3(t[1]), Alu.add)
                            nc.vector.tensor_tensor(cb(acc, 1), cb(acc, 1),
                                                    rv3(t[2]), Alu.add)
                            nc.vector.tensor_tensor(cb(acc, 1), cb(acc, 1),
                                                    rv3(t[3]), Alu.subtract)

                fetch_cim(0)
                prevB = None
                for c in range(ncoils):
                    if c + 1 < ncoils:
                        fetch_cim(c + 1)
                    A = make_A(c)
                    if prevB is None:
                        for g in A:
                            g()
                    else:
                        for gb, ga in zip(prevB, A):
                            gb()
                            ga()
                        emit_accum(c - 1)
                    prevB = make_B(c)
                for g in prevB:
                    g()
                emit_accum(ncoils - 1)

            def zdot(a_pl, b_pl, out_bc_tag):
                scr = imp.tile([P, FREEI], dt.float32, tag="im")
                part = scp.tile([P, 1], dt.float32, tag="zpart")
                nc.vector.scalar_tensor_tensor(scr[:], a_pl[:], 1.0, b_pl[:],
                                               Alu.mult, Alu.mult,
                                               accum_out=part[:])
                bc = scp.tile([P, 1], dt.float32, tag=out_bc_tag)
                nc.gpsimd.partition_all_reduce(bc[:], part[:], channels=P,
                                               reduce_op=bass_isa.ReduceOp.add)
                return bc

            def recip_eps(x_bc, tag):
                dn = scp.tile([P, 1], dt.float32, tag=tag + "_dn")
                nc.vector.tensor_scalar(dn[:], x_bc[:], EPS, None, Alu.add)
                inv = scp.tile([P, 1], dt.float32, tag=tag + "_inv")
                nc.vector.reciprocal(inv[:], dn[:])
                return inv

            # ---------- init:  rhs = adj + lam x ; r = rhs - Aop(rhs) ----------
            xs = imp.tile([P, FREEI], dt.float32, tag="im")
            as_ = imp.tile([P, FREEI], dt.float32, tag="im")
            for comp in (0, 1):
                for rb in range(NB):
                    sl = slice(comp * FREE + rb * W, comp * FREE + (rb + 1) * W)
                    nc.sync.dma_start(xs[:][:, sl], dram_comp(x_d[:], comp, rb))
                    nc.sync.dma_start(as_[:][:, sl], dram_comp(adj_d[:], comp, rb))
            nc.vector.scalar_tensor_tensor(xk[:], xs[:], LAM, as_[:],
                                           Alu.mult, Alu.add)       # xk = rhs
            emit_normal(xk)                                          # A^H A rhs
            nc.vector.scalar_tensor_tensor(acc[:], xk[:], LAM, acc[:],
                                           Alu.mult, Alu.add)       # Aop(rhs)
            nc.vector.tensor_tensor(r_pl[:], xk[:], acc[:], Alu.subtract)
            nc.vector.tensor_copy(p_pl[:], r_pl[:])                  # p = r
            rd_cur, rd_nxt = rd_a, rd_b
            scr0 = imp.tile([P, FREEI], dt.float32, tag="im")
            part0 = scp.tile([P, 1], dt.float32, tag="zpart")
            nc.vector.scalar_tensor_tensor(scr0[:], r_pl[:], 1.0, r_pl[:],
                                           Alu.mult, Alu.mult, accum_out=part0[:])
            nc.gpsimd.partition_all_reduce(rd_cur[:], part0[:], channels=P,
                                           reduce_op=bass_isa.ReduceOp.add)

            # ---------- CG iterations ----------
            for it in range(cg_iters):
                emit_normal(p_pl)
                nc.vector.scalar_tensor_tensor(acc[:], p_pl[:], LAM, acc[:],
                                               Alu.mult, Alu.add)    # acc = Ap
                pap_bc = zdot(p_pl, acc, "pap")
                inv = recip_eps(pap_bc, "pap")
                alpha = scp.tile([P, 1], dt.float32, tag="alpha")
                nc.vector.tensor_tensor(alpha[:], rd_cur[:], inv[:], Alu.mult)
                nalpha = scp.tile([P, 1], dt.float32, tag="nalpha")
                nc.vector.tensor_scalar(nalpha[:], alpha[:], -1.0, None, Alu.mult)
                with tc.high_priority(offset=-1_000_000):
                    nc.vector.scalar_tensor_tensor(xk[:], p_pl[:], alpha[:],
                                                   xk[:], Alu.mult, Alu.add)
                nc.vector.scalar_tensor_tensor(r_pl[:], acc[:], nalpha[:], r_pl[:],
                                               Alu.mult, Alu.add)
                scr1 = imp.tile([P, FREEI], dt.float32, tag="im")
                part1 = scp.tile([P, 1], dt.float32, tag="zpart")
                nc.vector.scalar_tensor_tensor(scr1[:], r_pl[:], 1.0, r_pl[:],
                                               Alu.mult, Alu.mult,
                                               accum_out=part1[:])
                nc.gpsimd.partition_all_reduce(rd_nxt[:], part1[:], channels=P,
                                               reduce_op=bass_isa.ReduceOp.add)
                invrd = recip_eps(rd_cur, "rd")
                beta = scp.tile([P, 1], dt.float32, tag="beta")
                nc.vector.tensor_tensor(beta[:], rd_nxt[:], invrd[:], Alu.mult)
                nc.vector.scalar_tensor_tensor(p_pl[:], p_pl[:], beta[:], r_pl[:],
                                               Alu.mult, Alu.add)
                rd_cur, rd_nxt = rd_nxt, rd_cur

            # comp-blocked sbuf -> interleaved dram
            for comp in (0, 1):
                for rb in range(NB):
                    sl = slice(comp * FREE + rb * W, comp * FREE + (rb + 1) * W)
                    nc.sync.dma_start(dram_comp(out_d[:], comp, rb), xk[:][:, sl])

    nc.compile()
    return nc


def _dft_mats():
    k = np.arange(H, dtype=np.float64)
    Wm = np.exp(-2j * np.pi * np.outer(k, k) / H) / np.sqrt(H)
    return Wm.real.astype(np.float32), Wm.imag.astype(np.float32)


def kernel(x, adj, maps, mask):
    from concourse.bass_utils import run_bass_kernel_spmd

    key = (CG_MAX, C, "f32r")
    if key not in _CACHE:
        _CACHE[key] = build(CG_MAX, C, "f32r")
    nc = _CACHE[key]
    gr, gi = _dft_mats()
    in_maps = []
    for b in range(B):
        in_maps.append({
            "x": np.ascontiguousarray(x[b], dtype=np.float32),
            "adj": np.ascontiguousarray(adj[b], dtype=np.float32),
            "maps": np.ascontiguousarray(maps[b], dtype=np.float32),
            "mask": np.ascontiguousarray(mask[b], dtype=np.float32),
            "gr": gr, "gi": gi,
        })
    res = run_bass_kernel_spmd(nc, in_maps, core_ids=list(range(B)))
    out = np.stack([res.results[b]["out"] for b in range(B)], axis=0)
    return out.astype(np.float32)


if __name__ == "__main__":
    nc = build()
    print("built + compiled ok")



# revision 13
# speedup vs baseline: 6.6989x; 6.6989x over previous
"""Trainium2 Bass kernel: SENSE-CG MRI reconstruction (nn_CG_module).

Problem: 10 CG iterations (plus initial residual) of
    (A^H A + lam I) x = adj + lam x0
where A = mask * FFT2 * coil-maps (12 coils, 384x384, ortho FFTs).

Strategy (per NeuronCore, pure batch-parallel over 8 cores):
  - 2D FFTs as dense DFT matmuls on the TensorEngine; the *data* is the
    stationary operand so each 1D stage yields the transposed partial for
    free; two stages restore orientation (zero transposes end to end).
  - All host-visible tensors are pre-laid-out on the HOST into
    component-blocked planes ([2,H,W], maps [C,2,H,W] fp16) so every DMA
    is a contiguous row gather; the output leaves comp-blocked and is
    re-interleaved on host.
  - fp16 for maps / DFT matrices / stage intermediates (rel err ~4e-4,
    gate 2e-2): doubles DVE elementwise throughput (2x_1p) and halves
    maps HBM traffic. CG state (xk/r/p/acc) stays fp32.
  - Coil loop streams maps from HBM, software-pipelined two ways: coil
    c+1's products are emitted before coil c's stages, and PE windows
    interleave coil c-1's (S3,S4) with coil c's (S1,S2) group-by-group.
  - CG scalar chain at iteration boundaries is split DVE/Pool, p is
    double-buffered so the xk axpy defers into the next normal op, and
    the dead tail ops of the final iteration are skipped.
"""
import numpy as np

B, C, H, W = 8, 12, 384, 384
LAM = 0.05
CG_MAX = 10
EPS = 1e-12
P = 128
NB = H // P            # 3 row blocks
FREE = NB * W          # 1152
FREEI = 2 * FREE       # 2304
SPL = 1536             # DVE | Pool split point for fp32 chain ops

_CACHE = {}


def build(cg_iters=CG_MAX, ncoils=C):
    import concourse.mybir as mybir
    import concourse.tile as tile
    import concourse.bass_isa as bass_isa
    from concourse import bacc

    dt = mybir.dt
    Alu = mybir.AluOpType
    MDT = dt.float16

    nc = bacc.Bacc("TRN2", target_bir_lowering=False, debug=False)

    x_d = nc.dram_tensor("x", [2, H, W], dt.float32, kind="ExternalInput")
    adj_d = nc.dram_tensor("adj", [2, H, W], dt.float32, kind="ExternalInput")
    maps_d = nc.dram_tensor("maps", [ncoils, 2, H, W], dt.float16,
                            kind="ExternalInput")
    mask_d = nc.dram_tensor("mask", [H, W], dt.float16, kind="ExternalInput")
    gr_d = nc.dram_tensor("gr", [H, W], dt.float16, kind="ExternalInput")
    gi_d = nc.dram_tensor("gi", [H, W], dt.float16, kind="ExternalInput")
    gin_d = nc.dram_tensor("gin", [H, W], dt.float16, kind="ExternalInput")
    out_d = nc.dram_tensor("out", [2, H, W], dt.float32, kind="ExternalOutput")

    def dram_cb(t):            # [2,H,W] (or [.,2,H,W]) -> [p, two, rb, w]
        return t.rearrange("two (rb p) w -> p two rb w", p=P)

    def sb_cb(t):              # comp-blocked plane tile -> [p, two, rb, w]
        return t[:].rearrange("p (two rb w) -> p two rb w", two=2, rb=NB)

    def dram_rv(t):            # [H,W] -> [p, rb, w]
        return t.rearrange("(rb p) w -> p rb w", p=P)

    def cb(t, comp):           # comp block of a plane tile -> [p, rb, w]
        return t[:][:, comp * FREE:(comp + 1) * FREE].rearrange(
            "p (rb w) -> p rb w", rb=NB)

    def rv3(t):                # real scratch tile -> [p, rb, w]
        return t[:].rearrange("p (rb w) -> p rb w", rb=NB)

    with tile.TileContext(nc) as tc:
        with (
            tc.tile_pool(name="state", bufs=1) as st,
            tc.tile_pool(name="gpool", bufs=1) as gp,
            tc.tile_pool(name="mapsp", bufs=4) as mp,
            tc.tile_pool(name="cimk", bufs=4) as ckp,
            tc.tile_pool(name="ws", bufs=3) as wp,
            tc.tile_pool(name="imp", bufs=2) as imp,
            tc.tile_pool(name="up", bufs=6) as up,
            tc.tile_pool(name="scr", bufs=2) as scrp,
            tc.tile_pool(name="scal", bufs=4) as scp,
            tc.tile_pool(name="ps", bufs=4, space="PSUM") as ps,
        ):
            # ---------- persistent state ----------
            xk = st.tile([P, FREEI], dt.float32, tag="xk")
            r_pl = st.tile([P, FREEI], dt.float32, tag="r")
            p_a = st.tile([P, FREEI], dt.float32, tag="p_a")
            p_b = st.tile([P, FREEI], dt.float32, tag="p_b")
            acc = st.tile([P, FREEI], dt.float32, tag="acc")
            vf16 = st.tile([P, FREEI], MDT, tag="vf16")
            gr_t = gp.tile([P, FREE], MDT, tag="gr")
            gi_t = gp.tile([P, FREE], MDT, tag="gi")
            gin_t = gp.tile([P, FREE], MDT, tag="gin")
            mask_t = gp.tile([P, FREE], MDT, tag="mask")
            rd_a = st.tile([P, 1], dt.float32, tag="rd_a")
            rd_b = st.tile([P, 1], dt.float32, tag="rd_b")
            nrd_a = st.tile([P, 1], dt.float32, tag="nrd_a")
            nrd_b = st.tile([P, 1], dt.float32, tag="nrd_b")

            # ---------- constants ----------
            nc.sync.dma_start(rv3(gr_t), dram_rv(gr_d[:]))
            nc.sync.dma_start(rv3(gi_t), dram_rv(gi_d[:]))
            nc.sync.dma_start(rv3(gin_t), dram_rv(gin_d[:]))
            nc.sync.dma_start(rv3(mask_t), dram_rv(mask_d[:]))

            # ---------- DFT stage emitter (dense 4-mul complex) ----------
            def stage(src, dst, fwd, out_mode):
                """dst[j,k] = sum_r src[r,j] * Gc[r,k]  (comp-blocked planes).

                fwd: Gc = G (forward DFT);  else Gc = conj(G).
                out_mode: 'plane' ACT copy to fp16, 'mask' DVE multiply by
                mask row-chunk. Returns NB closures (one per output group).
                """
                if fwd:
                    gA, gB, gC, gD = gr_t, gin_t, gi_t, gr_t
                else:
                    gA, gB, gC, gD = gr_t, gi_t, gin_t, gr_t
                dvb = dst[:].rearrange("p (two rb w) -> p two rb w",
                                       two=2, rb=NB)

                def emit_group(jb):
                    # r-half in psum bank 0 ([0:384]), i-half in bank 1
                    # ([512:896]); rb-major so the latest-arriving src chunk
                    # is consumed last and its latency is covered.
                    pt = ps.tile([P, 1024], dt.float32, tag="ps")
                    nhalf = {0: 0, 512: 0}
                    for rb in range(NB):
                        for off, comp, gx in ((0, 0, gA), (0, 1, gB),
                                              (512, 0, gC), (512, 1, gD)):
                            j = nhalf[off]
                            lhsT = src[:][:, comp * FREE + rb * W + jb * P:
                                          comp * FREE + rb * W + jb * P + P]
                            nc.tensor.matmul(
                                pt[:, off:off + W], lhsT,
                                gx[:, rb * W:(rb + 1) * W],
                                start=(j == 0), stop=(j == 2 * NB - 1))
                            nhalf[off] += 1
                    pin = pt[:].rearrange("p (two x) -> p two x", two=2)[:, :, 0:W]
                    dslice = dvb[:, :, jb, :]          # [p, 2, 384]
                    if out_mode == "mask":
                        mrow = mask_t[:, jb * W:(jb + 1) * W]
                        nc.vector.tensor_tensor(
                            dslice, pin,
                            mrow.unsqueeze(1).broadcast_to([P, 2, W]), Alu.mult)
                    else:
                        nc.scalar.copy(dslice, pin)

                return [lambda jb=jb: emit_group(jb) for jb in range(NB)]

            # ---------- SENSE normal operator: acc = A^H A v ----------
            def emit_normal(v16, defer=None):
                mbs = [None] * ncoils
                cims = [None] * ncoils
                kpls = [None] * ncoils
                ims = [None] * ncoils

                def fetch_cim(c, chunked=False):
                    mb = mp.tile([P, FREEI], MDT, tag="mapsbuf")
                    nc.sync.dma_start(sb_cb(mb), dram_cb(maps_d[:][c]))
                    cim = ckp.tile([P, FREEI], MDT, tag="cimk")
                    u0 = up.tile([P, FREE], MDT, tag="u")
                    u1 = up.tile([P, FREE], MDT, tag="u")
                    u2 = up.tile([P, FREE], MDT, tag="u")
                    u3 = up.tile([P, FREE], MDT, tag="u")
                    u = [u0, u1, u2, u3]
                    rbs = range(NB) if chunked else [None]
                    for rb in rbs:
                        if rb is None:
                            sl = slice(0, FREE)
                        else:
                            sl = slice(rb * W, (rb + 1) * W)
                        for ut, (ia, ib) in zip(u, ((0, 0), (1, 1),
                                                    (0, 1), (1, 0))):
                            nc.vector.tensor_tensor(
                                ut[:][:, sl],
                                mb[:][:, ia * FREE:][:, sl],
                                v16[:][:, ib * FREE:][:, sl], Alu.mult)
                        nc.vector.tensor_tensor(
                            cim[:][:, sl], u[0][:][:, sl], u[1][:][:, sl],
                            Alu.subtract)
                        nc.vector.tensor_tensor(
                            cim[:][:, FREE:][:, sl], u[2][:][:, sl],
                            u[3][:][:, sl], Alu.add)
                    mbs[c], cims[c] = mb, cim

                def make_A(c):
                    w1 = wp.tile([P, FREEI], MDT, tag="w1w3")
                    g1 = stage(cims[c], w1, fwd=True, out_mode="plane")
                    kpl = ckp.tile([P, FREEI], MDT, tag="cimk")
                    kpls[c] = kpl
                    g2 = stage(w1, kpl, fwd=True, out_mode="mask")
                    return g1 + g2

                def make_B(c):
                    w3 = wp.tile([P, FREEI], MDT, tag="w1w3")
                    g3 = stage(kpls[c], w3, fwd=False, out_mode="plane")
                    im = imp.tile([P, FREEI], MDT, tag="im")
                    ims[c] = im
                    g4 = stage(w3, im, fwd=False, out_mode="plane")
                    return g3 + g4

                def emit_accum(c, chunked=False):
                    # acc += conj(m)*im; fp16 pair-tree halves the fp32 adds.
                    # chunked (last coil): per-rb ops pipeline with the
                    # stage-4 group evacs and run at full priority so the
                    # boundary chain starts sooner.
                    mb, im = mbs[c], ims[c]
                    import contextlib
                    prio = (contextlib.nullcontext() if chunked
                            else tc.high_priority(offset=-1_000_000))
                    with prio:
                        t0_ = up.tile([P, FREE], MDT, tag="u")
                        t1_ = up.tile([P, FREE], MDT, tag="u")
                        t2_ = up.tile([P, FREE], MDT, tag="u")
                        t3_ = up.tile([P, FREE], MDT, tag="u")
                        s01 = up.tile([P, FREE], MDT, tag="u")
                        s23 = up.tile([P, FREE], MDT, tag="u")
                        t = [t0_, t1_, t2_, t3_]
                        for rb in (range(NB) if chunked else [None]):
                            sl = (slice(0, FREE) if rb is None
                                  else slice(rb * W, (rb + 1) * W))
                            for ut, (ia, ib) in zip(t, ((0, 0), (1, 1),
                                                        (0, 1), (1, 0))):
                                nc.vector.tensor_tensor(
                                    ut[:][:, sl],
                                    mb[:][:, ia * FREE:][:, sl],
                                    im[:][:, ib * FREE:][:, sl], Alu.mult)
                            nc.vector.tensor_tensor(s01[:][:, sl],
                                                    t[0][:][:, sl],
                                                    t[1][:][:, sl], Alu.add)
                            nc.vector.tensor_tensor(s23[:][:, sl],
                                                    t[2][:][:, sl],
                                                    t[3][:][:, sl],
                                                    Alu.subtract)
                            if c == 0:
                                nc.scalar.copy(acc[:][:, sl],
                                               s01[:][:, sl])
                                nc.scalar.copy(acc[:][:, FREE:][:, sl],
                                               s23[:][:, sl])
                            else:
                                nc.vector.tensor_tensor(
                                    acc[:][:, sl], acc[:][:, sl],
                                    s01[:][:, sl], Alu.add)
                                nc.gpsimd.tensor_tensor(
                                    acc[:][:, FREE:][:, sl],
                                    acc[:][:, FREE:][:, sl],
                                    s23[:][:, sl], Alu.add)

                fetch_cim(0, chunked=True)
                prevB = None
                for c in range(ncoils):
                    if c + 1 < ncoils:
                        fetch_cim(c + 1)
                    A = make_A(c)
                    if prevB is None:
                        for g in A:
                            g()
                    else:
                        for gb, ga in zip(prevB, A):
                            gb()
                            ga()
                        emit_accum(c - 1)
                        if c == 1 and defer is not None:
                            defer()
                    prevB = make_B(c)
                for g in prevB:
                    g()
                emit_accum(ncoils - 1, chunked=True)

            # ---------- fp32 chain helpers (DVE; STT is PE/DVE-only
            # on real V3 silicon, Pool rejects TensorScalarPtr) ----------
            def split_stt(out, in0, scalar, in1, op0, op1):
                nc.vector.scalar_tensor_tensor(
                    out[:], in0[:], scalar, in1[:], op0, op1)

            def split_dot(a_pl, b_pl, out_bc):
                """out_bc[p] = broadcast full sum of a*b (all partitions)."""
                scr = scrp.tile([P, FREEI], dt.float32, tag="scr")
                pa = scp.tile([P, 1], dt.float32, tag="zpa")
                nc.vector.scalar_tensor_tensor(
                    scr[:], a_pl[:], 1.0, b_pl[:], Alu.mult, Alu.mult,
                    accum_out=pa[:])
                nc.gpsimd.partition_all_reduce(out_bc[:], pa[:], channels=P,
                                               reduce_op=bass_isa.ReduceOp.add)

            def cast16(dst, src):
                """fp32 plane -> fp16 shadow; DVE comp0, Act comp1."""
                nc.vector.tensor_copy(dst[:][:, 0:FREE], src[:][:, 0:FREE])
                nc.scalar.copy(dst[:][:, FREE:], src[:][:, FREE:])

            # ---------- init:  rhs = adj + lam x ; r = rhs - Aop(rhs) -----
            xs = scrp.tile([P, FREEI], dt.float32, tag="scr")
            as_ = scrp.tile([P, FREEI], dt.float32, tag="scr")
            nc.sync.dma_start(sb_cb(xs), dram_cb(x_d[:]))
            nc.sync.dma_start(sb_cb(as_), dram_cb(adj_d[:]))
            split_stt(xk, xs, LAM, as_, Alu.mult, Alu.add)       # xk = rhs
            cast16(vf16, xk)
            emit_normal(vf16)                                    # A^H A rhs
            # r = (1-lam)*rhs - acc   (Aop(rhs) = acc + lam*rhs)
            split_stt(r_pl, xk, 1.0 - LAM, acc, Alu.mult, Alu.subtract)
            nc.vector.tensor_copy(p_a[:][:, 0:SPL], r_pl[:][:, 0:SPL])
            nc.scalar.copy(p_a[:][:, SPL:], r_pl[:][:, SPL:])
            cast16(vf16, r_pl)
            rd_cur, rd_nxt = rd_a, rd_b
            nrd_cur, nrd_nxt = nrd_a, nrd_b
            split_dot(r_pl, r_pl, rd_cur)
            nc.vector.tensor_scalar(nrd_cur[:], rd_cur[:], -1.0, None,
                                    Alu.mult)

            # ---------- CG iterations ----------
            p_cur, p_old = p_a, p_b
            pend = {"cb": None}

            def run_deferred():
                if pend["cb"] is not None:
                    cb_ = pend["cb"]
                    pend["cb"] = None
                    cb_()

            for it in range(cg_iters):
                last = it == cg_iters - 1
                emit_normal(vf16, defer=run_deferred)
                # acc <- Ap = acc + lam*p
                split_stt(acc, p_cur, LAM, acc, Alu.mult, Alu.add)
                pap_bc = scp.tile([P, 1], dt.float32, tag="pap")
                split_dot(p_cur, acc, pap_bc)
                dn = scp.tile([P, 1], dt.float32, tag="dn")
                nc.vector.tensor_scalar(dn[:], pap_bc[:], EPS, None, Alu.add)
                inv = scp.tile([P, 1], dt.float32, tag="inv")
                nc.vector.reciprocal(inv[:], dn[:])
                alpha = scp.tile([P, 1], dt.float32, tag="alpha")
                nc.vector.tensor_tensor(alpha[:], rd_cur[:], inv[:], Alu.mult)
                if last:
                    # only xk survives; update it at full priority and ship
                    split_stt(xk, p_cur, alpha[:], xk, Alu.mult, Alu.add)
                    nc.sync.dma_start(dram_cb(out_d[:]), sb_cb(xk))
                    break
                nalpha = scp.tile([P, 1], dt.float32, tag="nalpha")
                nc.vector.tensor_tensor(nalpha[:], nrd_cur[:], inv[:],
                                        Alu.mult)
                # invrd only needs rd_cur: compute before the scans so the
                # in-order DVE stream doesn't delay beta
                invrd = scp.tile([P, 1], dt.float32, tag="invrd")
                rdn = scp.tile([P, 1], dt.float32, tag="rdn")
                nc.vector.tensor_scalar(rdn[:], rd_cur[:], EPS, None, Alu.add)
                nc.vector.reciprocal(invrd[:], rdn[:])
                # r -= alpha*Ap   (critical path)
                split_stt(r_pl, acc, nalpha[:], r_pl, Alu.mult, Alu.add)
                split_dot(r_pl, r_pl, rd_nxt)
                nc.vector.tensor_scalar(nrd_nxt[:], rd_nxt[:], -1.0, None,
                                        Alu.mult)
                beta = scp.tile([P, 1], dt.float32, tag="beta")
                nc.vector.tensor_tensor(beta[:], rd_nxt[:], invrd[:], Alu.mult)
                # p_new = beta*p_old + r   into the other buffer
                split_stt(p_old, p_cur, beta[:], r_pl, Alu.mult, Alu.add)
                cast16(vf16, p_old)

                # deferred: xk += alpha*p_cur, emitted mid next normal op so
                # it never precedes the boundary chain in the DVE stream
                def mk_xk(alpha_t=alpha, p_t=p_cur):
                    def f():
                        nc.vector.scalar_tensor_tensor(
                            xk[:], p_t[:], alpha_t[:], xk[:],
                            Alu.mult, Alu.add)
                    return f
                pend["cb"] = mk_xk()
                p_cur, p_old = p_old, p_cur
                rd_cur, rd_nxt = rd_nxt, rd_cur
                nrd_cur, nrd_nxt = nrd_nxt, nrd_cur

    nc.compile()
    return nc


def _dft_mats():
    k = np.arange(H, dtype=np.float64)
    Wm = np.exp(-2j * np.pi * np.outer(k, k) / H) / np.sqrt(H)
    gr = Wm.real.astype(np.float16)
    gi = Wm.imag.astype(np.float16)
    gin = (-Wm.imag).astype(np.float16)
    return gr, gi, gin


def prep_in_maps(x, adj, maps, mask):
    """Host-side layout: comp-blocked planes, fp16 maps/mask/DFT mats."""
    gr, gi, gin = _dft_mats()
    in_maps = []
    for b in range(B):
        in_maps.append({
            "x": np.ascontiguousarray(
                np.moveaxis(x[b], -1, 0), dtype=np.float32),
            "adj": np.ascontiguousarray(
                np.moveaxis(adj[b], -1, 0), dtype=np.float32),
            "maps": np.ascontiguousarray(
                np.moveaxis(maps[b], -1, 1), dtype=np.float16),
            "mask": np.ascontiguousarray(mask[b], dtype=np.float16),
            "gr": gr, "gi": gi, "gin": gin,
        })
    return in_maps


def post_out(res):
    """[2,H,W] comp-blocked core outputs -> [B,H,W,2] fp32."""
    out = np.stack([np.moveaxis(res[b]["out"], 0, -1) for b in range(B)],
                   axis=0)
    return np.ascontiguousarray(out, dtype=np.float32)


def kernel(x, adj, maps, mask):
    from concourse.bass_utils import run_bass_kernel_spmd

    key = (CG_MAX, C)
    if key not in _CACHE:
        _CACHE[key] = build(CG_MAX, C)
    nc = _CACHE[key]
    in_maps = prep_in_maps(x, adj, maps, mask)
    res = run_bass_kernel_spmd(nc, in_maps, core_ids=list(range(B)))
    return post_out(res.results)


if __name__ == "__main__":
    nc = build()
    print("built + compiled ok")


# revision 14
# speedup vs baseline: 6.7080x; 1.0014x over previous
"""Trainium2 Bass kernel: SENSE-CG MRI reconstruction (nn_CG_module).

Problem: 10 CG iterations (plus initial residual) of
    (A^H A + lam I) x = adj + lam x0
where A = mask * FFT2 * coil-maps (12 coils, 384x384, ortho FFTs).

Strategy (per NeuronCore, pure batch-parallel over 8 cores):
  - 2D FFTs as dense DFT matmuls on the TensorEngine; the *data* is the
    stationary operand so each 1D stage yields the transposed partial for
    free; two stages restore orientation (zero transposes end to end).
  - All host-visible tensors are pre-laid-out on the HOST into
    component-blocked planes ([2,H,W], maps [C,2,H,W] fp16) so every DMA
    is a contiguous row gather; the output leaves comp-blocked and is
    re-interleaved on host.
  - fp16 for maps / DFT matrices / stage intermediates (rel err ~4e-4,
    gate 2e-2): doubles DVE elementwise throughput (2x_1p) and halves
    maps HBM traffic. CG state (xk/r/p/acc) stays fp32.
  - Coil loop streams maps from HBM, software-pipelined two ways: coil
    c+1's products are emitted before coil c's stages, and PE windows
    interleave coil c-1's (S3,S4) with coil c's (S1,S2) group-by-group.
  - CG scalar chain at iteration boundaries is split DVE/Pool, p is
    double-buffered so the xk axpy defers into the next normal op, and
    the dead tail ops of the final iteration are skipped.
"""
import numpy as np

B, C, H, W = 8, 12, 384, 384
LAM = 0.05
CG_MAX = 10
EPS = 1e-12
P = 128
NB = H // P            # 3 row blocks
FREE = NB * W          # 1152
FREEI = 2 * FREE       # 2304
SPL = 1536             # DVE | Pool split point for fp32 chain ops

_CACHE = {}


def build(cg_iters=CG_MAX, ncoils=C):
    import concourse.mybir as mybir
    import concourse.tile as tile
    import concourse.bass_isa as bass_isa
    from concourse import bacc

    dt = mybir.dt
    Alu = mybir.AluOpType
    MDT = dt.float16

    nc = bacc.Bacc("TRN2", target_bir_lowering=False, debug=False)

    x_d = nc.dram_tensor("x", [2, H, W], dt.float32, kind="ExternalInput")
    adj_d = nc.dram_tensor("adj", [2, H, W], dt.float32, kind="ExternalInput")
    maps_d = nc.dram_tensor("maps", [ncoils, 2, H, W], dt.float16,
                            kind="ExternalInput")
    mask_d = nc.dram_tensor("mask", [H, W], dt.float16, kind="ExternalInput")
    gr_d = nc.dram_tensor("gr", [H, W], dt.float16, kind="ExternalInput")
    gi_d = nc.dram_tensor("gi", [H, W], dt.float16, kind="ExternalInput")
    gin_d = nc.dram_tensor("gin", [H, W], dt.float16, kind="ExternalInput")
    out_d = nc.dram_tensor("out", [2, H, W], dt.float32, kind="ExternalOutput")

    def dram_cb(t):            # [2,H,W] (or [.,2,H,W]) -> [p, two, rb, w]
        return t.rearrange("two (rb p) w -> p two rb w", p=P)

    def sb_cb(t):              # comp-blocked plane tile -> [p, two, rb, w]
        return t[:].rearrange("p (two rb w) -> p two rb w", two=2, rb=NB)

    def dram_rv(t):            # [H,W] -> [p, rb, w]
        return t.rearrange("(rb p) w -> p rb w", p=P)

    def cb(t, comp):           # comp block of a plane tile -> [p, rb, w]
        return t[:][:, comp * FREE:(comp + 1) * FREE].rearrange(
            "p (rb w) -> p rb w", rb=NB)

    def rv3(t):                # real scratch tile -> [p, rb, w]
        return t[:].rearrange("p (rb w) -> p rb w", rb=NB)

    with tile.TileContext(nc) as tc:
        with (
            tc.tile_pool(name="state", bufs=1) as st,
            tc.tile_pool(name="gpool", bufs=1) as gp,
            tc.tile_pool(name="mapsp", bufs=4) as mp,
            tc.tile_pool(name="cimk", bufs=4) as ckp,
            tc.tile_pool(name="ws", bufs=3) as wp,
            tc.tile_pool(name="imp", bufs=2) as imp,
            tc.tile_pool(name="up", bufs=6) as up,
            tc.tile_pool(name="scr", bufs=2) as scrp,
            tc.tile_pool(name="scal", bufs=4) as scp,
            tc.tile_pool(name="ps", bufs=4, space="PSUM") as ps,
        ):
            # ---------- persistent state ----------
            xk = st.tile([P, FREEI], dt.float32, tag="xk")
            r_pl = st.tile([P, FREEI], dt.float32, tag="r")
            p_a = st.tile([P, FREEI], dt.float32, tag="p_a")
            p_b = st.tile([P, FREEI], dt.float32, tag="p_b")
            acc = st.tile([P, FREEI], dt.float32, tag="acc")
            np_t = st.tile([P, FREEI], dt.float32, tag="np_t")
            ap_t = st.tile([P, FREEI], dt.float32, tag="ap_t")
            vf16 = st.tile([P, FREEI], MDT, tag="vf16")
            gr_t = gp.tile([P, FREE], MDT, tag="gr")
            gi_t = gp.tile([P, FREE], MDT, tag="gi")
            gin_t = gp.tile([P, FREE], MDT, tag="gin")
            mask_t = gp.tile([P, FREE], MDT, tag="mask")
            rd_a = st.tile([P, 1], dt.float32, tag="rd_a")
            rd_b = st.tile([P, 1], dt.float32, tag="rd_b")
            nrd_a = st.tile([P, 1], dt.float32, tag="nrd_a")
            nrd_b = st.tile([P, 1], dt.float32, tag="nrd_b")

            # ---------- constants ----------
            nc.sync.dma_start(rv3(gr_t), dram_rv(gr_d[:]))
            nc.sync.dma_start(rv3(gi_t), dram_rv(gi_d[:]))
            nc.sync.dma_start(rv3(gin_t), dram_rv(gin_d[:]))
            nc.sync.dma_start(rv3(mask_t), dram_rv(mask_d[:]))

            # ---------- DFT stage emitter (dense 4-mul complex) ----------
            def stage(src, dst, fwd, out_mode):
                """dst[j,k] = sum_r src[r,j] * Gc[r,k]  (comp-blocked planes).

                fwd: Gc = G (forward DFT);  else Gc = conj(G).
                out_mode: 'plane' ACT copy to fp16, 'mask' DVE multiply by
                mask row-chunk. Returns NB closures (one per output group).
                """
                if fwd:
                    gA, gB, gC, gD = gr_t, gin_t, gi_t, gr_t
                else:
                    gA, gB, gC, gD = gr_t, gi_t, gin_t, gr_t
                dvb = dst[:].rearrange("p (two rb w) -> p two rb w",
                                       two=2, rb=NB)

                def emit_group(jb):
                    # r-half in psum bank 0 ([0:384]), i-half in bank 1
                    # ([512:896]); rb-major so the latest-arriving src chunk
                    # is consumed last and its latency is covered.
                    pt = ps.tile([P, 1024], dt.float32, tag="ps")
                    nhalf = {0: 0, 512: 0}
                    for rb in range(NB):
                        for off, comp, gx in ((0, 0, gA), (0, 1, gB),
                                              (512, 0, gC), (512, 1, gD)):
                            j = nhalf[off]
                            lhsT = src[:][:, comp * FREE + rb * W + jb * P:
                                          comp * FREE + rb * W + jb * P + P]
                            nc.tensor.matmul(
                                pt[:, off:off + W], lhsT,
                                gx[:, rb * W:(rb + 1) * W],
                                start=(j == 0), stop=(j == 2 * NB - 1))
                            nhalf[off] += 1
                    pin = pt[:].rearrange("p (two x) -> p two x", two=2)[:, :, 0:W]
                    dslice = dvb[:, :, jb, :]          # [p, 2, 384]
                    if out_mode == "mask":
                        mrow = mask_t[:, jb * W:(jb + 1) * W]
                        nc.vector.tensor_tensor(
                            dslice, pin,
                            mrow.unsqueeze(1).broadcast_to([P, 2, W]), Alu.mult)
                    else:
                        nc.scalar.copy(dslice, pin)

                return [lambda jb=jb: emit_group(jb) for jb in range(NB)]

            # ---------- SENSE normal operator: acc = A^H A v ----------
            def emit_normal(v16, defer=None):
                mbs = [None] * ncoils
                cims = [None] * ncoils
                kpls = [None] * ncoils
                ims = [None] * ncoils

                def fetch_cim(c, chunked=False):
                    mb = mp.tile([P, FREEI], MDT, tag="mapsbuf")
                    nc.sync.dma_start(sb_cb(mb), dram_cb(maps_d[:][c]))
                    cim = ckp.tile([P, FREEI], MDT, tag="cimk")
                    u0 = up.tile([P, FREE], MDT, tag="u")
                    u1 = up.tile([P, FREE], MDT, tag="u")
                    u2 = up.tile([P, FREE], MDT, tag="u")
                    u3 = up.tile([P, FREE], MDT, tag="u")
                    u = [u0, u1, u2, u3]
                    rbs = range(NB) if chunked else [None]
                    for rb in rbs:
                        if rb is None:
                            sl = slice(0, FREE)
                        else:
                            sl = slice(rb * W, (rb + 1) * W)
                        for ut, (ia, ib) in zip(u, ((0, 0), (1, 1),
                                                    (0, 1), (1, 0))):
                            nc.vector.tensor_tensor(
                                ut[:][:, sl],
                                mb[:][:, ia * FREE:][:, sl],
                                v16[:][:, ib * FREE:][:, sl], Alu.mult)
                        nc.vector.tensor_tensor(
                            cim[:][:, sl], u[0][:][:, sl], u[1][:][:, sl],
                            Alu.subtract)
                        nc.vector.tensor_tensor(
                            cim[:][:, FREE:][:, sl], u[2][:][:, sl],
                            u[3][:][:, sl], Alu.add)
                    mbs[c], cims[c] = mb, cim

                def make_A(c):
                    w1 = wp.tile([P, FREEI], MDT, tag="w1w3")
                    g1 = stage(cims[c], w1, fwd=True, out_mode="plane")
                    kpl = ckp.tile([P, FREEI], MDT, tag="cimk")
                    kpls[c] = kpl
                    g2 = stage(w1, kpl, fwd=True, out_mode="mask")
                    return g1 + g2

                def make_B(c):
                    w3 = wp.tile([P, FREEI], MDT, tag="w1w3")
                    g3 = stage(kpls[c], w3, fwd=False, out_mode="plane")
                    im = imp.tile([P, FREEI], MDT, tag="im")
                    ims[c] = im
                    g4 = stage(w3, im, fwd=False, out_mode="plane")
                    return g3 + g4

                def emit_accum(c, chunked=False):
                    # acc += conj(m)*im; fp16 pair-tree halves the fp32 adds.
                    # chunked (last coil): per-rb ops pipeline with the
                    # stage-4 group evacs and run at full priority so the
                    # boundary chain starts sooner.
                    mb, im = mbs[c], ims[c]
                    import contextlib
                    prio = (contextlib.nullcontext() if chunked
                            else tc.high_priority(offset=-1_000_000))
                    with prio:
                        t0_ = up.tile([P, FREE], MDT, tag="u")
                        t1_ = up.tile([P, FREE], MDT, tag="u")
                        t2_ = up.tile([P, FREE], MDT, tag="u")
                        t3_ = up.tile([P, FREE], MDT, tag="u")
                        s01 = up.tile([P, FREE], MDT, tag="u")
                        s23 = up.tile([P, FREE], MDT, tag="u")
                        t = [t0_, t1_, t2_, t3_]
                        for rb in (range(NB) if chunked else [None]):
                            sl = (slice(0, FREE) if rb is None
                                  else slice(rb * W, (rb + 1) * W))
                            for ut, (ia, ib) in zip(t, ((0, 0), (1, 1),
                                                        (0, 1), (1, 0))):
                                nc.vector.tensor_tensor(
                                    ut[:][:, sl],
                                    mb[:][:, ia * FREE:][:, sl],
                                    im[:][:, ib * FREE:][:, sl], Alu.mult)
                            nc.vector.tensor_tensor(s01[:][:, sl],
                                                    t[0][:][:, sl],
                                                    t[1][:][:, sl], Alu.add)
                            nc.vector.tensor_tensor(s23[:][:, sl],
                                                    t[2][:][:, sl],
                                                    t[3][:][:, sl],
                                                    Alu.subtract)
                            if c == 0:
                                nc.scalar.copy(acc[:][:, sl],
                                               s01[:][:, sl])
                                nc.scalar.copy(acc[:][:, FREE:][:, sl],
                                               s23[:][:, sl])
                            else:
                                nc.vector.tensor_tensor(
                                    acc[:][:, sl], acc[:][:, sl],
                                    s01[:][:, sl], Alu.add)
                                nc.gpsimd.tensor_tensor(
                                    acc[:][:, FREE:][:, sl],
                                    acc[:][:, FREE:][:, sl],
                                    s23[:][:, sl], Alu.add)

                fetch_cim(0, chunked=True)
                prevB = None
                for c in range(ncoils):
                    if c + 1 < ncoils:
                        fetch_cim(c + 1)
                    A = make_A(c)
                    if prevB is None:
                        for g in A:
                            g()
                    else:
                        for gb, ga in zip(prevB, A):
                            gb()
                            ga()
                        emit_accum(c - 1)
                        if c == 1 and defer is not None:
                            defer()
                    prevB = make_B(c)
                for g in prevB:
                    g()
                emit_accum(ncoils - 1, chunked=True)

            # ---------- fp32 chain helpers (DVE; STT is PE/DVE-only
            # on real V3 silicon, Pool rejects TensorScalarPtr) ----------
            def split_stt(out, in0, scalar, in1, op0, op1):
                nc.vector.scalar_tensor_tensor(
                    out[:], in0[:], scalar, in1[:], op0, op1)

            def split_dot(a_pl, b_pl, out_bc):
                """out_bc[p] = broadcast full sum of a*b (all partitions)."""
                scr = scrp.tile([P, FREEI], dt.float32, tag="scr")
                pa = scp.tile([P, 1], dt.float32, tag="zpa")
                nc.vector.scalar_tensor_tensor(
                    scr[:], a_pl[:], 1.0, b_pl[:], Alu.mult, Alu.mult,
                    accum_out=pa[:])
                nc.gpsimd.partition_all_reduce(out_bc[:], pa[:], channels=P,
                                               reduce_op=bass_isa.ReduceOp.add)

            def cast16(dst, src):
                """fp32 plane -> fp16 shadow; DVE comp0, Act comp1."""
                nc.vector.tensor_copy(dst[:][:, 0:FREE], src[:][:, 0:FREE])
                nc.scalar.copy(dst[:][:, FREE:], src[:][:, FREE:])

            # ---------- init:  rhs = adj + lam x ; r = rhs - Aop(rhs) -----
            xs = scrp.tile([P, FREEI], dt.float32, tag="scr")
            as_ = scrp.tile([P, FREEI], dt.float32, tag="scr")
            nc.sync.dma_start(sb_cb(xs), dram_cb(x_d[:]))
            nc.sync.dma_start(sb_cb(as_), dram_cb(adj_d[:]))
            split_stt(xk, xs, LAM, as_, Alu.mult, Alu.add)       # xk = rhs
            cast16(vf16, xk)
            emit_normal(vf16)                                    # A^H A rhs
            # r = (1-lam)*rhs - acc   (Aop(rhs) = acc + lam*rhs)
            split_stt(r_pl, xk, 1.0 - LAM, acc, Alu.mult, Alu.subtract)
            nc.vector.tensor_copy(p_a[:][:, 0:SPL], r_pl[:][:, 0:SPL])
            nc.scalar.copy(p_a[:][:, SPL:], r_pl[:][:, SPL:])
            cast16(vf16, r_pl)
            rd_cur, rd_nxt = rd_a, rd_b
            nrd_cur, nrd_nxt = nrd_a, nrd_b
            split_dot(r_pl, r_pl, rd_cur)
            nc.vector.tensor_scalar(nrd_cur[:], rd_cur[:], -1.0, None,
                                    Alu.mult)

            # ---------- CG iterations (normal-on-r recurrence) ----------
            # A^H A p_{k+1} = A^H A r_{k+1} + beta*(A^H A p_k), so the
            # normal op consumes r (ready right after the alpha update) and
            # rd/beta/p-update/xk all defer into the next normal op.
            p_cur, p_old = p_a, p_b
            pend = {"cb": None}

            def run_deferred():
                if pend["cb"] is not None:
                    cb_ = pend["cb"]
                    pend["cb"] = None
                    cb_()

            beta_prev = None
            for it in range(cg_iters):
                last = it == cg_iters - 1
                emit_normal(vf16, defer=run_deferred)   # acc = A^H A r_it
                if it == 0:
                    nc.vector.tensor_copy(np_t[:], acc[:])   # p_0 = r_0
                else:
                    nc.vector.scalar_tensor_tensor(
                        np_t[:], np_t[:], beta_prev[:], acc[:],
                        Alu.mult, Alu.add)
                # Ap = np_t + lam*p
                nc.vector.scalar_tensor_tensor(
                    ap_t[:], p_cur[:], LAM, np_t[:], Alu.mult, Alu.add)
                pap_bc = scp.tile([P, 1], dt.float32, tag="pap")
                split_dot(p_cur, ap_t, pap_bc)
                dn = scp.tile([P, 1], dt.float32, tag="dn")
                nc.vector.tensor_scalar(dn[:], pap_bc[:], EPS, None, Alu.add)
                inv = scp.tile([P, 1], dt.float32, tag="inv")
                nc.vector.reciprocal(inv[:], dn[:])
                alpha = scp.tile([P, 1], dt.float32, tag="alpha")
                nc.vector.tensor_tensor(alpha[:], rd_cur[:], inv[:], Alu.mult)
                if last:
                    # only xk survives; update at full priority and ship
                    split_stt(xk, p_cur, alpha[:], xk, Alu.mult, Alu.add)
                    nc.sync.dma_start(dram_cb(out_d[:]), sb_cb(xk))
                    break
                nalpha = scp.tile([P, 1], dt.float32, tag="nalpha")
                nc.vector.tensor_tensor(nalpha[:], nrd_cur[:], inv[:],
                                        Alu.mult)
                # r -= alpha*Ap  (critical), then hand f16(r) to the next
                # normal op immediately
                split_stt(r_pl, ap_t, nalpha[:], r_pl, Alu.mult, Alu.add)
                cast16(vf16, r_pl)

                # deferred tail, emitted mid next normal op: xk axpy, rd
                # scan, beta, p update
                beta_t = scp.tile([P, 1], dt.float32, tag="beta")

                def mk_tail(alpha_t=alpha, p_c=p_cur, p_o=p_old,
                            rd_c=rd_cur, rd_n=rd_nxt, nrd_n=nrd_nxt,
                            beta_o=beta_t):
                    def f():
                        nc.vector.scalar_tensor_tensor(
                            xk[:], p_c[:], alpha_t[:], xk[:],
                            Alu.mult, Alu.add)
                        split_dot(r_pl, r_pl, rd_n)
                        nc.vector.tensor_scalar(nrd_n[:], rd_n[:], -1.0,
                                                None, Alu.mult)
                        rdn2 = scp.tile([P, 1], dt.float32, tag="rdn")
                        nc.vector.tensor_scalar(rdn2[:], rd_c[:], EPS, None,
                                                Alu.add)
                        invrd2 = scp.tile([P, 1], dt.float32, tag="invrd")
                        nc.vector.reciprocal(invrd2[:], rdn2[:])
                        nc.vector.tensor_tensor(beta_o[:], rd_n[:],
                                                invrd2[:], Alu.mult)
                        nc.vector.scalar_tensor_tensor(
                            p_o[:], p_c[:], beta_o[:], r_pl[:],
                            Alu.mult, Alu.add)
                    return f

                pend["cb"] = mk_tail()
                beta_prev = beta_t
                p_cur, p_old = p_old, p_cur
                rd_cur, rd_nxt = rd_nxt, rd_cur
                nrd_cur, nrd_nxt = nrd_nxt, nrd_cur

    nc.compile()
    return nc


def _dft_mats():
    k = np.arange(H, dtype=np.float64)
    Wm = np.exp(-2j * np.pi * np.outer(k, k) / H) / np.sqrt(H)
    gr = Wm.real.astype(np.float16)
    gi = Wm.imag.astype(np.float16)
    gin = (-Wm.imag).astype(np.float16)
    return gr, gi, gin


def prep_in_maps(x, adj, maps, mask):
    """Host-side layout: comp-blocked planes, fp16 maps/mask/DFT mats."""
    gr, gi, gin = _dft_mats()
    in_maps = []
    for b in range(B):
        in_maps.append({
            "x": np.ascontiguousarray(
                np.moveaxis(x[b], -1, 0), dtype=np.float32),
            "adj": np.ascontiguousarray(
                np.moveaxis(adj[b], -1, 0), dtype=np.float32),
            "maps": np.ascontiguousarray(
                np.moveaxis(maps[b], -1, 1), dtype=np.float16),
            "mask": np.ascontiguousarray(mask[b], dtype=np.float16),
            "gr": gr, "gi": gi, "gin": gin,
        })
    return in_maps


def post_out(res):
    """[2,H,W] comp-blocked core outputs -> [B,H,W,2] fp32."""
    out = np.stack([np.moveaxis(res[b]["out"], 0, -1) for b in range(B)],
                   axis=0)
    return np.ascontiguousarray(out, dtype=np.float32)


def kernel(x, adj, maps, mask):
    from concourse.bass_utils import run_bass_kernel_spmd

    key = (CG_MAX, C)
    if key not in _CACHE:
        _CACHE[key] = build(CG_MAX, C)
    nc = _CACHE[key]
    in_maps = prep_in_maps(x, adj, maps, mask)
    res = run_bass_kernel_spmd(nc, in_maps, core_ids=list(range(B)))
    return post_out(res.results)


if __name__ == "__main__":
    nc = build()
    print("built + compiled ok")
